# revision 23
# baseline (speedup 1.0000x reference)
"""Trainium2 Bass kernel for nn_Loss_343597383760.

Loss:
    scores = predicted_values[rel_idx, e1_idx, e2_idx]        # [N] gather
    total  = sum(lab*sig(s) + (1-lab)*(1-sig(s))) = sum_i sigmoid(w_i * s_i)
    loss   = -total / ((1+neg)*N),  w = 2*lab-1, neg = #(lab==0)

Sharding (expert-style, per relation): core c owns relations {2c, 2c+1}.
The sign w is baked into the DRAM layout: each core gets
pv2 = [shard; -shard; 0.0] (f32, 2*TOTAL+1 elements); a triplet with
lab==0 indexes the negated half, so the device computes sigmoid(w*s)
directly with no weight tensor and no multiply. Pad slots index the
trailing 0.0 (sigmoid = 0.5, corrected on host).

Device pipeline per core: HWDGE (Sync) loads of the idx planes, then
per chunk a SWDGE indirect element gather g = pv2[idx] (one queue per
chunk; desc-gen is ~1.2us/instruction nearly independent of size, so
two big chunks win) feeding ACT sigmoid with accum_out into a raw SBUF
accumulator. The [128, NCHUNK] result store is issued OUTSIDE the
TileContext with a GpSimd completion gate: tile-tracked stores add a
~2.4us late-arming exit wait, while a completely unwaited store races
the NEFF teardown queue reset (loses data), so the raw store + explicit
gate is both the fast and the safe arrangement.
"""

import numpy as np

import concourse.bass as bass
import concourse.bacc as bacc
import concourse.tile as tile
from concourse import mybir
from concourse.bass_utils import run_bass_kernel_spmd

R, E, N = 16, 4096, 262144
NCORES = 8
RPC = R // NCORES            # relations per core
TOTAL = RPC * E * E          # elements in one core's shard
P = 128                      # SBUF partitions
CHUNK_COLS = [120, 144]      # columns per chunk; sum = 264
COLS = sum(CHUNK_COLS)
CAP = P * COLS               # 33792 (max bucket 33040)
NCHUNK = len(CHUNK_COLS)
GATHER_Q = [0, 1]            # SWDGE queue per gather chunk
NQ = 2
PAD_IDX = 2 * TOTAL          # index of the appended 0.0 element

# Set by test harness to capture a neuron-profile trace.
TRACE = False
LAST_RESULTS = None

_NC = None


def _indirect_gather_q(nc, out, in_, in_offset, queue_name):
    """indirect_dma_start with an explicit SWDGE queue (the stock API pins
    qPoolDynamic; separate queues let ring drains overlap desc-gen)."""
    orig = mybir.InstDMACopy

    def patched(**kw):
        kw["queue"] = queue_name
        return orig(**kw)

    mybir.InstDMACopy = patched
    try:
        return nc.gpsimd.indirect_dma_start(
            out=out, out_offset=None, in_=in_, in_offset=in_offset
        )
    finally:
        mybir.InstDMACopy = orig


def _build_nc():
    f32 = mybir.dt.float32
    i32 = mybir.dt.int32
    nc = bacc.Bacc(
        num_swdge_queues=NQ,
        enable_partition_id=False,
        monotonic_sem_count=0,
        dynamic_dma_scratch_size=65536,
    )
    pv = nc.declare_dram_parameter("pv", [2 * TOTAL + 1, 1], f32, isOutput=False)
    idxs = [
        nc.declare_dram_parameter(f"idx{k}", [P, c], i32, isOutput=False)
        for k, c in enumerate(CHUNK_COLS)
    ]
    out = nc.declare_dram_parameter("out", [P, NCHUNK], f32, isOutput=True)
    outbuf = nc.alloc_sbuf_tensor("outbuf", [P, NCHUNK], f32)

    with (
        tile.TileContext(nc) as tc,
        tc.tile_pool(name="io", bufs=NCHUNK) as io_pool,
        tc.tile_pool(name="work", bufs=NCHUNK) as work_pool,
    ):
        its = []
        for k, c in enumerate(CHUNK_COLS):
            it = io_pool.tile([P, c], i32, tag="idx")
            nc.sync.dma_start(out=it[:], in_=idxs[k][:])
            its.append(it)
        # Extra tracked DMA whose sem resolves mid-body: shifts the Tile
        # exit-wait pairing so the store sem gets its own (inline) wait,
        # avoiding a ~2us EVENT_SEMAPHORE wake stall on the idle Sync SEQ.
        dummy = io_pool.tile([P, 1], f32, tag="dummy")
        nc.sync.dma_start(out=dummy[:], in_=pv[0:P, :])
        for k, c in enumerate(CHUNK_COLS):
            g = work_pool.tile([P, c], f32, tag="gath")
            _indirect_gather_q(
                nc,
                out=g[:],
                in_=pv[:],
                in_offset=bass.IndirectOffsetOnAxis(ap=its[k][:], axis=0),
                queue_name=f"qPoolDynamic{GATHER_Q[k] or ''}",
            )
            sg = work_pool.tile([P, c], f32, tag="sig")
            nc.scalar.activation(
                out=sg[:],
                in_=g[:],
                func=mybir.ActivationFunctionType.Sigmoid,
                accum_out=outbuf.ap()[:, k : k + 1],
            )
    # Store OUTSIDE the TileContext: the tile exit-wait chain never blocks
    # on the store's HBM write receipt (each unmet exit wait costs a ~2.4us
    # event-arming stall). The tile-exit barrier orders this after the
    # accumulator writes. The GpSimd completion gate (fastest-arming engine)
    # holds the final all-engine barrier until the store lands, so the NEFF
    # teardown cannot reset the queue mid-flight.
    store_sem = nc.alloc_semaphore("raw_store_sem")
    nc.sync.dma_start(out=out[:], in_=outbuf.ap()).then_inc(store_sem, 16)
    nc.gpsimd.wait_ge(store_sem, 16)
    nc.finalize()
    return nc


def kernel(predicted_values, rel_idx, e1_idx, e2_idx, labels):
    global _NC, LAST_RESULTS
    pv = np.ascontiguousarray(np.asarray(predicted_values, dtype=np.float32))
    rel = np.asarray(rel_idx, dtype=np.int64)
    e1 = np.asarray(e1_idx, dtype=np.int64)
    e2 = np.asarray(e2_idx, dtype=np.int64)
    lab = np.asarray(labels, dtype=np.int64)

    owner = rel // RPC
    # flat element index into the signed local shard: negated half for lab==0
    local_flat = (rel % RPC) * (E * E) + e1 * E + e2 + (lab == 0) * TOTAL
    pv_flat = pv.reshape(R * E * E)

    host_extra = 0.0   # sum of sigmoid(w*s) for overflow triplets (host-computed)
    pad_total = 0.0    # total pad slots across cores (each contributes 0.5)
    in_maps = []
    for c in range(NCORES):
        fi = local_flat[owner == c]
        if fi.size > CAP:
            of = fi[CAP:]
            s = pv_flat[np.where(of >= TOTAL, of - TOTAL, of) + c * TOTAL].astype(
                np.float64
            )
            s = np.where(of >= TOTAL, -s, s)
            host_extra += float(np.sum(1.0 / (1.0 + np.exp(-s))))
            fi = fi[:CAP]
        pad_total += float(CAP - fi.size)
        idx_arr = np.full(CAP, PAD_IDX, np.int32)
        idx_arr[: fi.size] = fi.astype(np.int32)
        shard = pv[c * RPC : (c + 1) * RPC].reshape(TOTAL)
        pv2 = np.empty(2 * TOTAL + 1, np.float32)
        pv2[:TOTAL] = shard
        pv2[TOTAL : 2 * TOTAL] = -shard
        pv2[2 * TOTAL] = 0.0
        m = {"pv": pv2.reshape(2 * TOTAL + 1, 1)}
        off = 0
        for k, ck in enumerate(CHUNK_COLS):
            m[f"idx{k}"] = idx_arr[off : off + P * ck].reshape(P, ck)
            off += P * ck
        in_maps.append(m)

    if _NC is None:
        _NC = _build_nc()

    res = run_bass_kernel_spmd(
        _NC, in_maps, core_ids=list(range(NCORES)), trace=TRACE
    )
    LAST_RESULTS = res

    # device sums sigmoid(w*s) per slot; pads contribute sigmoid(0) = 0.5
    total = host_extra - 0.5 * pad_total
    for c in range(NCORES):
        total += float(np.asarray(res.results[c]["out"], dtype=np.float64).sum())

    neg = float(np.sum(lab == 0))
    loss = -total / ((1.0 + neg) * float(N))
    return np.array([loss], dtype=np.float32)


# revision 25
# speedup vs baseline: 1.2129x; 1.2129x over previous
"""Trainium2 Bass kernel for nn_Loss_343597383760.

Loss:
    scores = predicted_values[rel_idx, e1_idx, e2_idx]        # [N] gather
    total  = sum(lab*sig(s) + (1-lab)*(1-sig(s))) = sum_i sigmoid(w_i * s_i)
    loss   = -total / ((1+neg)*N),  w = 2*lab-1, neg = #(lab==0)

Sharding (expert-style, per relation): core c owns relations {2c, 2c+1}.
The sign w is baked into the DRAM layout: each core gets
pv2 = [shard; -shard; 0.0] (f32, 2*TOTAL+1 elements); a triplet with
lab==0 indexes the negated half, so the device computes sigmoid(w*s)
directly with no weight tensor and no multiply. Pad slots index the
trailing 0.0 (sigmoid = 0.5, corrected on host).

Device pipeline per core: HWDGE (Sync) loads of the idx planes, then
per chunk a SWDGE indirect element gather g = pv2[idx] (one queue per
chunk; desc-gen is ~1.2us/instruction nearly independent of size, so
two big chunks win) feeding ACT sigmoid with accum_out into a raw SBUF
accumulator, then a tile-tracked [128, NCHUNK] store on Sync. An early
dummy Sync DMA shifts the tile exit-wait pairing so fewer exit waits
dispatch with unmet conditions (each such wait costs a ~2.4us
event-arming stall).
"""

import numpy as np

import concourse.bass as bass
import concourse.bacc as bacc
import concourse.tile as tile
from concourse import mybir
from concourse.bass_utils import run_bass_kernel_spmd

R, E, N = 16, 4096, 262144
NCORES = 8
RPC = R // NCORES            # relations per core
TOTAL = RPC * E * E          # elements in one core's shard
P = 128                      # SBUF partitions
CHUNK_COLS = [120, 144]      # columns per chunk; sum = 264
COLS = sum(CHUNK_COLS)
CAP = P * COLS               # 33792 (max bucket 33040)
NCHUNK = len(CHUNK_COLS)
GATHER_Q = [0, 1]            # SWDGE queue per gather chunk
NQ = 2
PAD_IDX = 2 * TOTAL          # index of the appended 0.0 element

# Set by test harness to capture a neuron-profile trace.
TRACE = False
LAST_RESULTS = None

_NC = None


def _indirect_gather_q(nc, out, in_, in_offset, queue_name):
    """indirect_dma_start with an explicit SWDGE queue (the stock API pins
    qPoolDynamic; separate queues let ring drains overlap desc-gen)."""
    orig = mybir.InstDMACopy

    def patched(**kw):
        kw["queue"] = queue_name
        return orig(**kw)

    mybir.InstDMACopy = patched
    try:
        return nc.gpsimd.indirect_dma_start(
            out=out, out_offset=None, in_=in_, in_offset=in_offset
        )
    finally:
        mybir.InstDMACopy = orig


def _build_nc():
    f32 = mybir.dt.float32
    i32 = mybir.dt.int32
    nc = bacc.Bacc(
        num_swdge_queues=NQ,
        enable_partition_id=False,
        monotonic_sem_count=0,
        dynamic_dma_scratch_size=65536,
    )
    pv = nc.declare_dram_parameter("pv", [2 * TOTAL + 1, 1], f32, isOutput=False)
    idxs = [
        nc.declare_dram_parameter(f"idx{k}", [P, c], i32, isOutput=False)
        for k, c in enumerate(CHUNK_COLS)
    ]
    out = nc.declare_dram_parameter("out", [P, NCHUNK], f32, isOutput=True)
    outbuf = nc.alloc_sbuf_tensor("outbuf", [P, NCHUNK], f32)

    with (
        tile.TileContext(nc) as tc,
        tc.tile_pool(name="io", bufs=NCHUNK) as io_pool,
        tc.tile_pool(name="work", bufs=NCHUNK) as work_pool,
    ):
        its = []
        for k, c in enumerate(CHUNK_COLS):
            it = io_pool.tile([P, c], i32, tag="idx")
            nc.sync.dma_start(out=it[:], in_=idxs[k][:])
            its.append(it)
        # Extra tracked DMA whose sem resolves mid-body: shifts the Tile
        # exit-wait pairing so the store sem gets its own (inline) wait,
        # avoiding a ~2us EVENT_SEMAPHORE wake stall on the idle Sync SEQ.
        dummy = io_pool.tile([P, 1], f32, tag="dummy")
        nc.sync.dma_start(out=dummy[:], in_=pv[0:P, :])
        for k, c in enumerate(CHUNK_COLS):
            g = work_pool.tile([P, c], f32, tag="gath")
            _indirect_gather_q(
                nc,
                out=g[:],
                in_=pv[:],
                in_offset=bass.IndirectOffsetOnAxis(ap=its[k][:], axis=0),
                queue_name=f"qPoolDynamic{GATHER_Q[k] or ''}",
            )
            sg = work_pool.tile([P, c], f32, tag="sig")
            nc.scalar.activation(
                out=sg[:],
                in_=g[:],
                func=mybir.ActivationFunctionType.Sigmoid,
                accum_out=outbuf.ap()[:, k : k + 1],
            )
        # Tile-tracked store on Sync: the tile exit waits for its completion
        # sem before any teardown drain runs. (A store outside the tile with
        # only a manual gate intermittently loses data to the teardown's
        # queue reset - observed rel_err jumping 13-20x - so the tracked
        # store is mandatory for correctness despite its ~2.4us exit-wait
        # arming cost.)
        nc.sync.dma_start(out=out[:], in_=outbuf.ap())
    nc.finalize()
    return nc


def kernel(predicted_values, rel_idx, e1_idx, e2_idx, labels):
    global _NC, LAST_RESULTS
    pv = np.ascontiguousarray(np.asarray(predicted_values, dtype=np.float32))
    rel = np.asarray(rel_idx, dtype=np.int64)
    e1 = np.asarray(e1_idx, dtype=np.int64)
    e2 = np.asarray(e2_idx, dtype=np.int64)
    lab = np.asarray(labels, dtype=np.int64)

    owner = rel // RPC
    # flat element index into the signed local shard: negated half for lab==0
    local_flat = (rel % RPC) * (E * E) + e1 * E + e2 + (lab == 0) * TOTAL
    pv_flat = pv.reshape(R * E * E)

    host_extra = 0.0   # sum of sigmoid(w*s) for overflow triplets (host-computed)
    pad_total = 0.0    # total pad slots across cores (each contributes 0.5)
    in_maps = []
    for c in range(NCORES):
        fi = local_flat[owner == c]
        if fi.size > CAP:
            of = fi[CAP:]
            s = pv_flat[np.where(of >= TOTAL, of - TOTAL, of) + c * TOTAL].astype(
                np.float64
            )
            s = np.where(of >= TOTAL, -s, s)
            host_extra += float(np.sum(1.0 / (1.0 + np.exp(-s))))
            fi = fi[:CAP]
        pad_total += float(CAP - fi.size)
        idx_arr = np.full(CAP, PAD_IDX, np.int32)
        idx_arr[: fi.size] = fi.astype(np.int32)
        shard = pv[c * RPC : (c + 1) * RPC].reshape(TOTAL)
        pv2 = np.empty(2 * TOTAL + 1, np.float32)
        pv2[:TOTAL] = shard
        pv2[TOTAL : 2 * TOTAL] = -shard
        pv2[2 * TOTAL] = 0.0
        m = {"pv": pv2.reshape(2 * TOTAL + 1, 1)}
        off = 0
        for k, ck in enumerate(CHUNK_COLS):
            m[f"idx{k}"] = idx_arr[off : off + P * ck].reshape(P, ck)
            off += P * ck
        in_maps.append(m)

    if _NC is None:
        _NC = _build_nc()

    res = run_bass_kernel_spmd(
        _NC, in_maps, core_ids=list(range(NCORES)), trace=TRACE
    )
    LAST_RESULTS = res

    # device sums sigmoid(w*s) per slot; pads contribute sigmoid(0) = 0.5
    total = host_extra - 0.5 * pad_total
    for c in range(NCORES):
        total += float(np.asarray(res.results[c]["out"], dtype=np.float64).sum())

    neg = float(np.sum(lab == 0))
    loss = -total / ((1.0 + neg) * float(N))
    return np.array([loss], dtype=np.float32)
